# revision 14
# baseline (speedup 1.0000x reference)
"""Trainium2 Bass kernel for nn_ModelNew_78847009620052 (dense_mlp).

Computes, for x [4096, 8192] and weight [8192, 8192]:
    out[b, 0] = 0.75 * sum_i x[b, i] * (sum_j weight[j, i])
(which equals 1.5 * sum(x @ W.T / 2, axis=1, keepdims=True)).

Sharding: column-shard the contraction dim IN=8192 into 8 chunks of 1024.
Core d receives x[:, d*1024:(d+1)*1024] and weight[:, d*1024:(d+1)*1024],
produces a partial [4096, 1]; host sums the 8 partials.

Per-core device algorithm (memory-bound: 48MB of input per core; the
per-pair HBM roofline is ~716 GB/s shared by 2 cores, so the target is
DMA-engine occupancy ~100% with w streamed strictly before x):

  All loads are issued on nc.sync (SP HWDGE) -> one FIFO descriptor ring,
  so the byte order on the wire is exactly the issue order. All w/x tiles
  share ONE tile-pool ring (tag "stream", 8 x 2MB bufs): each allocation's
  DMA can only be issued once the buffer's previous occupant was consumed,
  which (a) paces issue, (b) guarantees x DMAs enqueue after all w DMAs,
  so the weight stream gets full bandwidth until it completes, then x
  streams seamlessly while VectorE consumes it.

  Phase 1 (w, 32MB): 15 x 2MB tiles ([128, 4096] via "(p t) c -> p (t c)",
  16KB contiguous per-partition descriptors), then 1MB + 2 x 0.5MB pieces
  (the taper shortens the dependency tail between the last w byte and the
  finished column sums). Per 2MB tile, VectorE tree-adds the 4 row-groups
  (2 ops), then TensorE matmuls the [128, 1024] partial against a
  stationary 0.75-constant [128, 128] (reduces over partitions AND
  broadcasts the scaled column sums to all 128 PSUM partitions; 0.75
  folds the reference's /2 * 1.5). The 0.5MB pieces skip VectorE and are
  matmul'd raw. PSUM accumulates everything into psum_bc [128, 1024].

  Phase 2 (x, 16MB): 7 x 2MB tiles ([128, 4, 1024] via "(t p) c -> p t c"
  so tile slice t holds batch rows 512i+128t..+127 in partition order),
  then 1MB + 2 x 0.5MB (taper again). Each [128, 1024] slice is consumed
  by ONE VectorE scalar_tensor_tensor op reading the column sums straight
  from PSUM: out = (x * 1.0) * psum_bc, accum_out = row sums -> s_sbuf
  column n (n = global 128-row group index). Tile n covers batch rows
  128n..128n+127, so s_sbuf[p, n] = out[128n + p].

  Finish: s_sbuf [128, 32] is transposed on TensorE ([32, 128] in PSUM),
  copied to SBUF on ScalarE, and stored as one contiguous 16KB DMA.
"""

import numpy as np

B, IN, HID = 4096, 8192, 8192
N_CORES = 8
CHUNK = IN // N_CORES          # 1024 columns per core
SCALE = 1.5 / 2.0              # 0.75, folded into the ones matrix
P = 128                        # partitions
N_GROUPS = B // P              # 32 x row-groups per core

_compiled_nc = None


def _build_nc():
    import concourse.bass as bass
    import concourse.tile as tile
    from concourse import bacc, mybir
    from concourse.masks import make_identity

    f32 = mybir.dt.float32
    nc = bacc.Bacc(
        "TRN2",
        target_bir_lowering=False,
        debug=False,
        num_devices=N_CORES,
    )

    x_d = nc.dram_tensor("x", [B, CHUNK], f32, kind="ExternalInput")
    w_d = nc.dram_tensor("w", [HID, CHUNK], f32, kind="ExternalInput")
    out_d = nc.dram_tensor("out", [B, 1], f32, kind="ExternalOutput")

    with tile.TileContext(nc) as tc:
        with (
            tc.tile_pool(name="stream", bufs=11) as stream,
            tc.tile_pool(name="scratch", bufs=2) as scratch,
            tc.tile_pool(name="const", bufs=1) as const,
            tc.tile_pool(name="psum", bufs=1, space="PSUM") as psum_pool,
        ):
            ones = const.tile([P, P], f32)
            nc.vector.memset(ones[:], SCALE)
            identity = const.tile([P, P], f32)
            make_identity(nc, identity)
            s_sbuf = const.tile([P, N_GROUPS], f32)
            sT = const.tile([N_GROUPS, P], f32)
            sTB = const.tile([4, P], f32)

            psum_bc = psum_pool.tile([P, CHUNK], f32, tag="psum_bc")
            psum_tA = psum_pool.tile([28, P], f32, tag="psum_tA")
            psum_tB = psum_pool.tile([4, P], f32, tag="psum_tB")
            psum_junk = psum_pool.tile([P, P], f32, tag="psum_junk")

            def w_matmuls(src_ap, start, stop):
                for h in range(2):
                    nc.tensor.matmul(
                        psum_bc[:, h * 512 : (h + 1) * 512],
                        ones[:],
                        src_ap[:, h * 512 : (h + 1) * 512],
                        start=start,
                        stop=stop,
                    )

            # --- Phase 1: stream w, accumulate 0.75 * column sums ---
            # DVE-only accumulation: per 2MB tile, two [128, 2048] adds fold
            # the tile into a persistent accumulator. Ring buffers are
            # released by DVE alone -- no cross-engine feedback loop (in
            # v2-v5, ring WAR deps on PE matmuls locked the whole pipeline to
            # the DMA cadence with a standing ~14us phase lag). The
            # partition-reduce + broadcast matmuls run once, at the end.
            acc = const.tile([P, 2, CHUNK], f32)
            for j in range(15):  # 2MB tiles, rows 512j .. 512j+511
                wl = stream.tile([P, 4, CHUNK], f32, tag="stream")
                nc.sync.dma_start(
                    wl[:],
                    w_d[j * 512 : (j + 1) * 512, :].rearrange(
                        "(p t) c -> p (t c)", p=P
                    ),
                )
                if j == 0:
                    nc.vector.tensor_add(acc[:], wl[:, 0:2, :], wl[:, 2:4, :])
                else:
                    nc.vector.tensor_add(
                        wl[:, 0:2, :], wl[:, 0:2, :], wl[:, 2:4, :]
                    )
                    nc.vector.tensor_add(acc[:], acc[:], wl[:, 0:2, :])
                # Data-paced dummy transpose keeps TensorE's clock ramped so
                # the final matmuls run warm; reads a slice the adds don't
                # write, so it joins no dependency chain except the DMA.
                ndum = 4 if j >= 13 else 1
                for k in range(ndum):
                    nc.tensor.transpose(
                        psum_junk[:], wl[:, 2, k * P : (k + 1) * P], identity[:]
                    )

            # 1MB piece, rows 7680..7935 -> accumulator
            pw = stream.tile([P, 4, CHUNK], f32, tag="stream")
            nc.sync.dma_start(
                pw[:, 0:2, :],
                w_d[7680:7936, :].rearrange("(p t) c -> p (t c)", p=P),
            )
            nc.vector.tensor_add(acc[:], acc[:], pw[:, 0:2, :])

            # two 0.5MB pieces matmul'd raw (their matmuls run while the acc
            # merge finishes), then the accumulator's matmul pair closes the
            # PSUM group. Tail after the last w byte: ~2 warm matmul pairs.
            q1 = stream.tile([P, 4, CHUNK], f32, tag="stream")
            nc.sync.dma_start(q1[:, 0, :], w_d[7936:8064, :])
            q2 = stream.tile([P, 4, CHUNK], f32, tag="stream")
            nc.sync.dma_start(q2[:, 0, :], w_d[8064:8192, :])
            nc.vector.tensor_add(acc[:, 0, :], acc[:, 0, :], acc[:, 1, :])
            w_matmuls(q1[:, 0, :], start=True, stop=False)
            w_matmuls(q2[:, 0, :], start=False, stop=False)
            w_matmuls(acc[:, 0, :], start=False, stop=True)

            # --- Phase 2: stream x, fused multiply+row-sum on VectorE ---
            def x_op(xl, t, n):
                scr = scratch.tile([P, CHUNK], f32, tag="scr")
                nc.vector.scalar_tensor_tensor(
                    out=scr[:],
                    in0=xl[:, t, :],
                    scalar=1.0,
                    in1=psum_bc[:],
                    op0=mybir.AluOpType.mult,
                    op1=mybir.AluOpType.mult,
                    accum_out=s_sbuf[:, n : n + 1],
                )

            # 1MB piece first (rows 3584..3839, n = 28, 29): it lands before
            # psum_bc closes, so the fused-op chain starts as early as psum_bc
            # allows instead of waiting for a full 2MB tile.
            xl = stream.tile([P, 4, CHUNK], f32, tag="stream")
            nc.sync.dma_start(
                xl[:, 0:2, :],
                x_d[3584:3840, :].rearrange("(t p) c -> p t c", p=P),
            )
            x_op(xl, 0, 28)
            x_op(xl, 1, 29)

            for i in range(7):  # 2MB tiles, rows 512i .. 512i+511
                xl = stream.tile([P, 4, CHUNK], f32, tag="stream")
                nc.sync.dma_start(
                    xl[:],
                    x_d[i * 512 : (i + 1) * 512, :].rearrange(
                        "(t p) c -> p t c", p=P
                    ),
                )
                for t in range(4):
                    x_op(xl, t, 4 * i + t)

            # two 0.5MB pieces (n = 30, 31)
            for k, (r0, r1) in enumerate([(3840, 3968), (3968, 4096)]):
                xl = stream.tile([P, 4, CHUNK], f32, tag="stream")
                nc.sync.dma_start(xl[:, 0, :], x_d[r0:r1, :])
                x_op(xl, 0, 30 + k)

            # --- Finish: transpose s to [32, 128] and store contiguously.
            # Piece A (rows 0..3583) finalizes while the x tail streams; the
            # stores issue on the ACT HWDGE ring so they bypass the SP ring's
            # FIFO (whose tail is still draining x bytes).
            nc.tensor.transpose(psum_tA[:], s_sbuf[:, 0:28], identity[:])
            nc.scalar.copy(sT[0:28, :], psum_tA[:])
            nc.scalar.dma_start(
                out_d[0:3584].rearrange("(n p) o -> n (p o)", p=P), sT[0:28, :]
            )
            nc.tensor.transpose(psum_tB[:], s_sbuf[:, 28:32], identity[:])
            nc.scalar.copy(sTB[:], psum_tB[:])
            nc.scalar.dma_start(
                out_d[3584:4096].rearrange("(n p) o -> n (p o)", p=P), sTB[:]
            )

    nc.compile()
    return nc


def _get_nc():
    global _compiled_nc
    if _compiled_nc is None:
        _compiled_nc = _build_nc()
    return _compiled_nc


def kernel(x: np.ndarray, weight: np.ndarray) -> np.ndarray:
    from concourse.bass_utils import run_bass_kernel_spmd

    x = np.asarray(x, dtype=np.float32)
    weight = np.asarray(weight, dtype=np.float32)
    assert x.shape == (B, IN) and weight.shape == (HID, IN)

    nc = _get_nc()
    in_maps = [
        {
            "x": np.ascontiguousarray(x[:, d * CHUNK : (d + 1) * CHUNK]),
            "w": np.ascontiguousarray(weight[:, d * CHUNK : (d + 1) * CHUNK]),
        }
        for d in range(N_CORES)
    ]
    res = run_bass_kernel_spmd(nc, in_maps, core_ids=list(range(N_CORES)))
    acc = np.zeros((B, 1), dtype=np.float64)
    for d in range(N_CORES):
        acc += res.results[d]["out"].astype(np.float64)
    return acc.astype(np.float32)
